# revision 3
# baseline (speedup 1.0000x reference)
"""Trainium2 Bass kernel for nn_LinearNNEncoder (fused Linear+GELU, masked per-batch
mean/std over ragged sequences), data-parallel over 8 NeuronCores.

Contract: kernel(**inputs) takes FULL inputs (x [64,2048,300] f32, W [300,300],
b [300]) and returns the FULL output [64, 600] f32 (concat(std, mean) per batch).

Design (v4):
  - Host drops padding rows and packs the global valid-token stream
    contiguously into 128-token slots split evenly across 8 cores (batches
    may span slot/core boundaries; the host epilogue re-combines per-batch
    sums). C = ceil(total/1024) slots per core (79 for this input).
  - Contraction trimmed to K=302 (300 dims + ones/bias + ones/bias-residual)
    in fp8 DoubleRowSwInterleave: k0 = dims 0..255 (128 partitions), k1 =
    dims 256..301 (23 partitions, replicated across 4 PE quadrant bands
    0/32/64/96 so its DMA payload spreads over all 128 partitions).
    2 matmuls/slot, cost ~62.5ns each (scales with the 300-wide output).
  - GELU on ACT in 3-slot batches from a 3-bank PSUM tile (the wall:
    0.833ns/elem + 185ns/op access latency); DVE squares each triple in one
    2x-mode op. ACT runs gap-free through the whole steady state.
  - Per-slot stats (sum y, sum y^2 over tokens) routed by slot index: ~55%
    to Pool (one 600-wide partition_all_reduce per slot -> SBUF rows), rest
    to PE (two 125ns one-hot matmuls accumulating start=False into two
    pinned PSUM banks, partition = local batch group). The per-slot [128,M]
    one-hot is host DATA, so batch boundaries inside a slot work under the
    SPMD single-program constraint; the host also permutes slots so boundary
    slots land on PE-routed indices (Pool needs single-batch slots). Stats
    lag their gelu by 4 triples to decouple the in-order PE queue from the
    ACT->DVE->stats chain.
  - Loads ride the SP queue (v1 cost model: free-dim bytes x 0.386ns, 2x
    below 512B chunks, ~500ns/DMA floor, +1717ns completion latency): x0 in
    4-pair groups, k1 as one wide banded tensor, prologue (w + first pairs
    + first k1 columns) as two early tiles. Drains overlap the tail: pool
    rows via the gpsimd queue right after their last op; pe stats via two
    parallel 500ns DMAs (SP + ACT) fed by DVE/ACT copies of the pinned
    banks. The last 6 slots are PE-routed so no Pool op trails the last act.
  - Host epilogue (float64): combine sums per batch, torch.std_mean
    semantics (unbiased, n==1 -> std=0), NaN->0.
  - Measured: 32,189 ns (CoreSim cost model; baseline 41,493), HW rel err
    1.19e-2 (gate 2e-2).
"""
import numpy as np
import ml_dtypes

B, T, D = 64, 2048, 300
NCORES = 8
P = 128
K1P = 23            # k1 chunk partitions (dims 256..301 as 23x2)

_cache = {}


def _routing(C, M):
    """Fixed per-program routing: which slot indices go to Pool vs PE.
    The last 6 slots are PE-routed so every Pool op (and the pool_stats
    drain) finishes well before the pipeline tail."""
    Ce = C - 6
    npool = int(round(Ce * 0.55))
    pool_idx = []
    pe_idx = []
    acc = 0
    for s in range(C):
        nacc = ((min(s, Ce - 1) + 1) * npool) // Ce if s < Ce else acc
        if nacc > acc:
            pool_idx.append(s)
        else:
            pe_idx.append(s)
        acc = nacc
    return pool_idx, pe_idx


def _build_nc(C, M, NPOOL):
    from contextlib import ExitStack
    import concourse.tile as tile
    from concourse import mybir, bacc, bass_isa, library_config

    f32 = mybir.dt.float32
    bf16 = mybir.dt.bfloat16
    fp8 = mybir.dt.float8e4
    AF = mybir.ActivationFunctionType
    PM = mybir.MatmulPerfMode

    pool_idx, pe_idx = _routing(C, M)
    assert len(pool_idx) == NPOOL
    NPE = C - NPOOL
    route = {}
    for i, s in enumerate(pool_idx):
        route[s] = ("pool", i)
    for i, s in enumerate(pe_idx):
        route[s] = ("pe", i)

    C2 = (C + 1) // 2
    NT = (C + 2) // 3
    KB = -(-C2 // 4)    # k1 columns: col k holds pairs 4k..4k+3 on 4 bands

    nc = bacc.Bacc("TRN2", target_bir_lowering=False, debug=False)
    x0_dram = nc.dram_tensor("x0", [C2, P, 512], fp8, kind="ExternalInput")
    x1_dram = nc.dram_tensor("x1", [P, KB, 512], fp8, kind="ExternalInput")
    wxa_dram = nc.dram_tensor("wxa", [P, 1624], fp8, kind="ExternalInput")
    wxb_dram = nc.dram_tensor("wxb", [P, 1624], fp8, kind="ExternalInput")
    oh_dram = nc.dram_tensor("oh", [P, NPE, M], bf16, kind="ExternalInput")
    pool_dram = nc.dram_tensor("pool_stats", [NPOOL, 2 * D], bf16,
                               kind="ExternalOutput")
    pey_dram = nc.dram_tensor("pe_y", [M, D], f32, kind="ExternalOutput")
    pey2_dram = nc.dram_tensor("pe_y2", [M, D], f32, kind="ExternalOutput")

    # load groups in pairs-of-slots (pairs 0-1 ride the head wx DMA);
    # small first groups for fast pipeline fill
    sizes = [2, 2, 4]
    while sum(sizes) < C2 - 2:
        sizes.append(min(4, C2 - 2 - sum(sizes)))
    while sum(sizes) > C2 - 2:
        t = sizes.pop()
        excess = sum(sizes) + t - (C2 - 2)
        if t - excess > 0:
            sizes.append(t - excess)
    starts = [2 + sum(sizes[:gi]) for gi in range(len(sizes))]
    g_of_pair = {}
    for gi, w in enumerate(sizes):
        for q in range(w):
            g_of_pair[starts[gi] + q] = gi
    LEADP = 12           # pairs of load lead

    with ExitStack() as ctx:
        tc = ctx.enter_context(tile.TileContext(nc))
        const = ctx.enter_context(tc.tile_pool(name="const", bufs=1))
        # pinned stats banks FIRST -> PSUM banks 0,1
        ps_pin = ctx.enter_context(tc.tile_pool(name="ps_pin", bufs=1, space="PSUM"))
        ps_y = ctx.enter_context(tc.tile_pool(name="ps_y", bufs=2, space="PSUM"))
        xtp = ctx.enter_context(tc.tile_pool(name="xtp", bufs=6))
        ysp = ctx.enter_context(tc.tile_pool(name="ysp", bufs=8))
        stq = ctx.enter_context(tc.tile_pool(name="stq", bufs=1))

        nc.gpsimd.load_library(library_config.attn)

        # prologue tiles: A = [0:600 w_k0 | 600:1624 x0 pairs 0-1]
        #                 B = [0:600 w_k1 | 600:1624 k1 cols 0-1]
        wxa_sb = const.tile([P, 1624], fp8)
        wxb_sb = const.tile([P, 1624], fp8)
        oh_sb = const.tile([P, NPE, M], bf16)
        xt1_sb = const.tile([P, KB, 512], fp8)  # k1: 4 bands at 0/32/64/96

        pinY = ps_pin.tile([P, 512], f32, name="pinY", tag="pinY")
        pinY2 = ps_pin.tile([P, 512], f32, name="pinY2", tag="pinY2")
        nc.vector.memset(pinY[0:M, 0:D], 0.0)
        nc.vector.memset(pinY2[0:M, 0:D], 0.0)

        sall = stq.tile([P, NPOOL, 2 * D], bf16, name="sall", tag="sall")
        stg = stq.tile([P, 2 * D], f32, name="stg", tag="stg")
        nc.sync.dma_start(wxa_sb[:], wxa_dram.ap())
        nc.sync.dma_start(wxb_sb[:], wxb_dram.ap())

        stL = {}          # load group -> xt0 tile
        stY = {}          # triple -> ys tile
        next_load = 0

        def issue_loads(up_to_pair):
            nonlocal next_load
            while next_load < len(sizes) and starts[next_load] <= up_to_pair:
                gi = next_load
                w = sizes[gi]
                q0 = starts[gi]
                xt0 = xtp.tile([P, 4, 512], fp8, name=f"xt0_{gi}", tag="xt0")
                nc.sync.dma_start(
                    xt0[:, 0:w, :],
                    x0_dram.ap()[q0:q0 + w].rearrange("q p c -> p q c"),
                )
                stL[gi] = xt0
                next_load += 1
                if gi == 1:
                    nc.sync.dma_start(oh_sb[:], oh_dram.ap())
                elif gi == 2 and KB > 2:
                    nc.sync.dma_start(xt1_sb[:, 2:KB, :],
                                      x1_dram.ap()[:, 2:KB, :])

        def mains(t):
            w3 = min(3, C - 3 * t)
            y3 = ps_y.tile([P, 1536], f32, name=f"y3_{t}", tag="y3")
            for si in range(w3):
                s = 3 * t + si
                q, sub = s // 2, s % 2
                if q < 2:
                    st_ap = wxa_sb[:, 600 + 512 * q + 256 * sub:
                                   600 + 512 * q + 256 * sub + 256]
                else:
                    gi = g_of_pair[q]
                    xt0 = stL[gi]
                    ql = q - starts[gi]
                    st_ap = xt0[:, ql, 256 * sub:256 * sub + 256]
                nc.tensor.matmul(
                    y3[:, 512 * si:512 * si + D],
                    st_ap.rearrange("p (j i) -> p j i", i=2),
                    wxa_sb[:, 0:600].rearrange("p (k n) -> p k n", n=D),
                    start=True, stop=False, perf_mode=PM.DoubleRowSwInterleave,
                )
            for si in range(w3):
                s = 3 * t + si
                q, sub = s // 2, s % 2
                bo = 32 * (q % 4)
                col = q // 4
                if col < 2:
                    st1_ap = wxb_sb[bo:bo + K1P,
                                    600 + 512 * col + 256 * sub:
                                    600 + 512 * col + 256 * sub + 256]
                else:
                    st1_ap = xt1_sb[bo:bo + K1P, col, 256 * sub:256 * sub + 256]
                nc.tensor.matmul(
                    y3[:, 512 * si:512 * si + D],
                    st1_ap.rearrange("p (j i) -> p j i", i=2),
                    wxb_sb[bo:bo + K1P, 0:600]
                        .rearrange("p (k n) -> p k n", n=D),
                    start=False, stop=True, perf_mode=PM.DoubleRowSwInterleave,
                    tile_position=(bo, 0),
                )
            return y3

        def actdve(t, y3):
            w3 = min(3, C - 3 * t)
            ys = ysp.tile([P, 3, 2 * D], bf16, name=f"ys_{t}", tag="ys")
            nc.scalar.activation(
                ys[:, 0:w3, 0:D],
                y3[:].rearrange("p (s c) -> p s c", s=3)[:, 0:w3, 0:D],
                AF.Gelu,
            )
            nc.vector.tensor_mul(
                ys[:, 0:w3, D:2 * D], ys[:, 0:w3, 0:D], ys[:, 0:w3, 0:D]
            )
            stY[t] = ys

        def stats(t):
            w3 = min(3, C - 3 * t)
            ys = stY.pop(t)
            pe_sis = []
            for si in range(w3):
                s = 3 * t + si
                kind, i = route[s]
                if kind == "pool":
                    nc.gpsimd.partition_all_reduce(
                        sall[:, i, :], ys[:, si, :],
                        channels=P, reduce_op=bass_isa.ReduceOp.add,
                    )
                else:
                    pe_sis.append((si, i))
            for si, i in pe_sis:
                nc.tensor.matmul(
                    pinY[0:M, 0:D], oh_sb[:, i, 0:M], ys[:, si, 0:D],
                    start=False, stop=True, skip_group_check=True,
                )
            for si, i in pe_sis:
                nc.tensor.matmul(
                    pinY2[0:M, 0:D], oh_sb[:, i, 0:M], ys[:, si, D:2 * D],
                    start=False, stop=True, skip_group_check=True,
                )

        LAG = 4           # triples between gelu and its stats (decouples the
                          # PE in-order queue from the ACT->DVE->stats chain)
        y3_of = {}
        for t in range(NT + LAG):
            if t < NT:
                lastpair = min(3 * t + 2 + 2 * LEADP, C - 1) // 2
                issue_loads(lastpair)
                y3_of[t] = mains(t)
            if 1 <= t <= NT:
                actdve(t - 1, y3_of.pop(t - 1))
            if t >= LAG:
                stats(t - LAG)
                if t - LAG == NT - 3:
                    # all Pool ops done (none in the last 2 triples)
                    nc.gpsimd.dma_start(pool_dram.ap(), sall[0:1, :, :])

        # copyY on DVE, copyY2 on the (now idle) ACT engine, in parallel;
        # then two 500ns-floor drains in parallel on SP and ACT
        nc.vector.tensor_copy(stg[0:M, 0:D], pinY[0:M, 0:D])
        nc.scalar.activation(stg[0:M, D:2 * D], pinY2[0:M, 0:D], AF.Copy)
        nc.sync.dma_start(pey_dram.ap(), stg[0:M, 0:D])
        nc.scalar.dma_start(pey2_dram.ap(), stg[0:M, D:2 * D])

    nc.compile()
    return nc


def _pack_inputs(x, W, b, M=16):
    """Host prep. Returns (arrays..., meta) for all cores."""
    f8 = ml_dtypes.float8_e4m3
    bff = ml_dtypes.bfloat16
    x = np.asarray(x, np.float32)
    # padding rows are all -1.0; checking the first 8 dims is exact in
    # practice (P[gaussian row starts with 8 exact -1.0s] ~ 1e-56)
    valid = ~np.all(x[:, :, :8] == -1.0, axis=-1)    # [B, T]
    n = valid.sum(axis=1).astype(np.int64)           # [B]
    total = int(n.sum())

    C = -(-total // (NCORES * P))
    percore = C * P
    padtot = NCORES * percore

    toks = np.zeros((padtot, 302), np.float32)
    toks[:total, :D] = x[valid]
    toks[:total, D] = 1.0       # bias column
    toks[:total, D + 1] = 1.0   # bias fp8-residual column
    btok = np.full(padtot, -1, np.int64)
    btok[:total] = np.repeat(np.arange(B), n)

    pool_idx, pe_idx = _routing(C, M)
    NPOOL, NPE = len(pool_idx), len(pe_idx)

    Cp = C + (C & 1)               # data padded to even slots for pairs
    C2 = Cp // 2
    KB = -(-C2 // 4)
    x0 = np.zeros((NCORES, C2, P, 512), f8)
    x1 = np.zeros((NCORES, P, KB, 512), f8)
    oh = np.zeros((NCORES, P, NPE, M), bff)
    pool_map = np.full((NCORES, NPOOL), -1, np.int64)
    group_map = np.full((NCORES, M), -1, np.int64)

    for c in range(NCORES):
        ct = toks[c * percore:(c + 1) * percore].reshape(C, P, 302)
        cb = btok[c * percore:(c + 1) * percore].reshape(C, P)

        # local batch groups in order of appearance
        gids = {}
        for bid in cb.reshape(-1):
            if bid >= 0 and bid not in gids:
                gids[bid] = len(gids)
        G = len(gids)
        assert G <= M, f"core {c}: {G} batch groups > M={M}"
        for bid, g in gids.items():
            group_map[c, g] = bid

        # slot classification: poolable = at most one real batch in the slot
        nb_per_slot = [np.unique(cb[s][cb[s] >= 0]) for s in range(C)]
        boundary = [s for s in range(C) if len(nb_per_slot[s]) > 1]
        single = [s for s in range(C) if len(nb_per_slot[s]) <= 1]
        assert len(boundary) <= NPE, f"core {c}: too many boundary slots"

        # permutation: old slot -> new index. Boundary slots must land on
        # PE-routed indices.
        old_of_new = np.empty(C, np.int64)
        pe_free = list(pe_idx)
        for s in boundary:
            old_of_new[pe_free.pop(0)] = s
        rest = single
        fill = pool_idx + pe_free
        fill.sort()
        for pos, s in zip(fill, rest):
            old_of_new[pos] = s

        # stationaries (built in old order, then permuted)
        if Cp != C:
            ct = np.concatenate([ct, np.zeros((1, P, 302), np.float32)], 0)
        ctq = ct.astype(f8)
        a = ctq[:, :, 0:256].reshape(Cp, P, 2, 128)
        st0 = np.ascontiguousarray(
            a.transpose(0, 3, 1, 2)[:, :, ::-1, :]).reshape(Cp, P, 256)
        bb = ctq[:, :, 256:302].reshape(Cp, P, K1P, 2)
        st1 = np.ascontiguousarray(
            bb.transpose(0, 2, 1, 3)[:, :, ::-1, :]).reshape(Cp, K1P, 256)
        perm = np.concatenate([old_of_new, np.arange(C, Cp)])
        st0 = st0[perm]
        st1 = st1[perm]
        x0[c] = st0.reshape(C2, 2, P, 256).transpose(0, 2, 1, 3).reshape(
            C2, P, 512)
        # k1 stationaries pair-packed (512B chunks), pair q at partition
        # band 32*(q%4), column q//4 -> all 128 partitions carry payload
        st1p = st1.reshape(C2, 2, K1P, 256).transpose(0, 2, 1, 3).reshape(
            C2, K1P, 512)                                  # [q, p, 512]
        for bq in range(C2):
            x1[c, 32 * (bq % 4):32 * (bq % 4) + K1P, bq // 4, :] = st1p[bq]

        # metadata + one-hots in new order
        for i, pos in enumerate(pool_idx):
            s = old_of_new[pos]
            bs = nb_per_slot[s]
            if len(bs):
                pool_map[c, i] = bs[0]
        for i, pos in enumerate(pe_idx):
            s = old_of_new[pos]
            sb = cb[s]
            for t_ in range(P):
                if sb[t_] >= 0:
                    oh[c, t_, i, gids[sb[t_]]] = 1.0

    wtf = np.zeros((302, D), np.float32)
    wtf[:D, :] = np.asarray(W, np.float32).T
    bf = np.asarray(b, np.float32)
    b8 = bf.astype(f8).astype(np.float32)
    wtf[D, :] = b8                 # fp8-rounded bias
    wtf[D + 1, :] = bf - b8        # residual, cancels bias quantization
    wq = wtf.astype(f8)
    wcomb = np.zeros((P, 4, D), f8)
    wcomb[:, 0:2, :] = wq[0:256].reshape(2, 128, D).transpose(1, 0, 2)
    for b_ in range(4):
        wcomb[32 * b_:32 * b_ + K1P, 2:4, :] = wq[256:302].reshape(K1P, 2, D)
    wxa = np.zeros((NCORES, P, 1624), f8)
    wxa[:, :, 0:600] = wcomb[:, 0:2, :].reshape(1, P, 600)
    wxa[:, :, 600:1624] = x0[:, 0:2].transpose(0, 2, 1, 3).reshape(
        NCORES, P, 1024)
    wxb = np.zeros((NCORES, P, 1624), f8)
    wxb[:, :, 0:600] = wcomb[:, 2:4, :].reshape(1, P, 600)
    wxb[:, :, 600:1624] = x1[:, :, 0:2, :].reshape(NCORES, P, 1024)

    meta = (C, M, NPOOL, pool_map, group_map, n)
    return x0, x1, wxa, wxb, oh, meta


def _epilogue(pool_stats, pe_stats, meta):
    """pool_stats [NC, NPOOL, 600], pe_stats [NC, M, 600] -> out [B, 600]."""
    C, M, NPOOL, pool_map, group_map, n = meta
    acc = np.zeros((B + 1, 2 * D), np.float64)
    np.add.at(acc, np.where(pool_map < 0, B, pool_map).reshape(-1),
              pool_stats.reshape(-1, 2 * D).astype(np.float64))
    np.add.at(acc, np.where(group_map < 0, B, group_map).reshape(-1),
              pe_stats.reshape(-1, 2 * D).astype(np.float64))
    sy = acc[:B, 0:D]
    sy2 = acc[:B, D:2 * D]
    nf = n.astype(np.float64)[:, None]
    with np.errstate(divide="ignore", invalid="ignore"):
        mean = sy / nf
        var = (sy2 - nf * mean * mean) / np.maximum(nf - 1.0, 1.0)
        std = np.where(nf > 1.0, np.sqrt(np.maximum(var, 0.0)), 0.0)
    out = np.concatenate([std, mean], axis=-1)
    out = np.where(np.isnan(out), 0.0, out)
    return out.astype(np.float32)


def _get_nc(C, M, NPOOL):
    key = ("nc", C, M, NPOOL)
    if key not in _cache:
        _cache[key] = _build_nc(C, M, NPOOL)
    return _cache[key]


def _prep(x, W, b):
    for M in (16, 32, 64, 128):
        try:
            x0, x1, wxa, wxb, oh, meta = _pack_inputs(x, W, b, M=M)
            break
        except AssertionError:
            continue
    C, M, NPOOL = meta[0], meta[1], meta[2]
    nc = _get_nc(C, M, NPOOL)
    in_maps = [
        {"x0": x0[c], "x1": x1[c], "wxa": wxa[c], "wxb": wxb[c],
         "oh": np.asarray(oh[c])}
        for c in range(NCORES)
    ]
    return nc, in_maps, meta


def kernel(x, W, b):
    from concourse.bass_utils import run_bass_kernel_spmd

    nc, in_maps, meta = _prep(x, W, b)
    res = run_bass_kernel_spmd(nc, in_maps, core_ids=list(range(NCORES)))
    M = meta[1]
    pool_stats = np.stack([res.results[c]["pool_stats"] for c in range(NCORES)])
    pe_stats = np.stack([
        np.concatenate([res.results[c]["pe_y"], res.results[c]["pe_y2"]],
                       axis=-1) for c in range(NCORES)])
    return _epilogue(pool_stats.astype(np.float64),
                     pe_stats.astype(np.float64), meta)


def sim_prep(x, W, b):
    """Hook for sim_time.py: returns (nc, in_maps); caches meta for sim_check."""
    nc, in_maps, meta = _prep(x, W, b)
    _cache["meta"] = meta
    return nc, in_maps


def sim_check(sim, ins, expected):
    """Hook for sim_time.py: rel err over batches fully on core 0."""
    meta = _cache["meta"]
    C, M, NPOOL, pool_map, group_map, n = meta
    pool_stats = np.zeros((NCORES, NPOOL, 2 * D), np.float64)
    pe_stats = np.zeros((NCORES, M, 2 * D), np.float64)
    pool_stats[0] = np.asarray(sim.tensor("pool_stats")).astype(np.float64)
    pe_stats[0] = np.concatenate(
        [np.asarray(sim.tensor("pe_y")).astype(np.float64),
         np.asarray(sim.tensor("pe_y2")).astype(np.float64)], axis=-1)
    out = _epilogue(pool_stats, pe_stats, meta)
    # batches entirely inside core 0's token window
    bs = sorted(set(int(v) for v in pool_map[0] if v >= 0)
                | set(int(v) for v in group_map[0] if v >= 0))
    others = set(int(v) for v in pool_map[1:].reshape(-1) if v >= 0) \
        | set(int(v) for v in group_map[1:].reshape(-1) if v >= 0)
    bs = [b_ for b_ in bs if b_ not in others]
    return np.abs(out[bs] - expected[bs]).max() / np.abs(expected).max()


# revision 4
# speedup vs baseline: 1.0019x; 1.0019x over previous
"""Trainium2 Bass kernel for nn_LinearNNEncoder (fused Linear+GELU, masked per-batch
mean/std over ragged sequences), data-parallel over 8 NeuronCores.

Contract: kernel(**inputs) takes FULL inputs (x [64,2048,300] f32, W [300,300],
b [300]) and returns the FULL output [64, 600] f32 (concat(std, mean) per batch).

Design (v4):
  - Host drops padding rows and packs the global valid-token stream
    contiguously into 128-token slots split evenly across 8 cores (batches
    may span slot/core boundaries; the host epilogue re-combines per-batch
    sums). C = ceil(total/1024) slots per core (79 for this input).
  - Contraction trimmed to K=302 (300 dims + ones/bias + ones/bias-residual)
    in fp8 DoubleRowSwInterleave: k0 = dims 0..255 (128 partitions), k1 =
    dims 256..301 (23 partitions, replicated across 4 PE quadrant bands
    0/32/64/96 so its DMA payload spreads over all 128 partitions).
    2 matmuls/slot, cost ~62.5ns each (scales with the 300-wide output).
  - GELU on ACT in 3-slot batches from a 3-bank PSUM tile (the wall:
    0.833ns/elem + 185ns/op access latency); DVE squares each triple in one
    2x-mode op. ACT runs gap-free through the whole steady state.
  - Per-slot stats (sum y, sum y^2 over tokens) routed by slot index: ~55%
    to Pool (one 600-wide partition_all_reduce per slot -> SBUF rows), rest
    to PE (two 125ns one-hot matmuls accumulating start=False into two
    pinned PSUM banks, partition = local batch group). The per-slot [128,M]
    one-hot is host DATA, so batch boundaries inside a slot work under the
    SPMD single-program constraint; the host also permutes slots so boundary
    slots land on PE-routed indices (Pool needs single-batch slots). Stats
    lag their gelu by 4 triples to decouple the in-order PE queue from the
    ACT->DVE->stats chain.
  - Loads ride the SP queue (v1 cost model: free-dim bytes x 0.386ns, 2x
    below 512B chunks, ~500ns/DMA floor, +1717ns completion latency): x0 in
    4-pair groups, k1 as one wide banded tensor, prologue (w + first pairs
    + first k1 columns) as two early tiles. Drains overlap the tail: pool
    rows via the gpsimd queue right after their last op; pe stats via two
    parallel 500ns DMAs (SP + ACT) fed by DVE/ACT copies of the pinned
    banks. The last 6 slots are PE-routed so no Pool op trails the last act.
  - Host epilogue (float64): combine sums per batch, torch.std_mean
    semantics (unbiased, n==1 -> std=0), NaN->0.
  - Measured: 32,127 ns (CoreSim cost model; baseline 41,493), HW rel err
    1.19e-2 (gate 2e-2).
"""
import numpy as np
import ml_dtypes

B, T, D = 64, 2048, 300
NCORES = 8
P = 128
K1P = 23            # k1 chunk partitions (dims 256..301 as 23x2)

_cache = {}


def _routing(C, M):
    """Fixed per-program routing: which slot indices go to Pool vs PE.
    The last 6 slots are PE-routed so every Pool op (and the pool_stats
    drain) finishes well before the pipeline tail."""
    Ce = C - 6
    npool = int(round(Ce * 0.55))
    pool_idx = []
    pe_idx = []
    acc = 0
    for s in range(C):
        nacc = ((min(s, Ce - 1) + 1) * npool) // Ce if s < Ce else acc
        if nacc > acc:
            pool_idx.append(s)
        else:
            pe_idx.append(s)
        acc = nacc
    return pool_idx, pe_idx


def _build_nc(C, M, NPOOL):
    from contextlib import ExitStack
    import concourse.tile as tile
    from concourse import mybir, bacc, bass_isa, library_config

    f32 = mybir.dt.float32
    bf16 = mybir.dt.bfloat16
    fp8 = mybir.dt.float8e4
    AF = mybir.ActivationFunctionType
    PM = mybir.MatmulPerfMode

    pool_idx, pe_idx = _routing(C, M)
    assert len(pool_idx) == NPOOL
    NPE = C - NPOOL
    route = {}
    for i, s in enumerate(pool_idx):
        route[s] = ("pool", i)
    for i, s in enumerate(pe_idx):
        route[s] = ("pe", i)

    C2 = (C + 1) // 2
    NT = (C + 2) // 3
    KB = -(-C2 // 4)    # k1 columns: col k holds pairs 4k..4k+3 on 4 bands

    nc = bacc.Bacc("TRN2", target_bir_lowering=False, debug=False)
    x0_dram = nc.dram_tensor("x0", [C2, P, 512], fp8, kind="ExternalInput")
    x1_dram = nc.dram_tensor("x1", [P, KB, 512], fp8, kind="ExternalInput")
    wxa_dram = nc.dram_tensor("wxa", [P, 1624], fp8, kind="ExternalInput")
    wxb_dram = nc.dram_tensor("wxb", [P, 1624], fp8, kind="ExternalInput")
    oh_dram = nc.dram_tensor("oh", [P, NPE, M], bf16, kind="ExternalInput")
    pool_dram = nc.dram_tensor("pool_stats", [NPOOL, 2 * D], bf16,
                               kind="ExternalOutput")
    pey_dram = nc.dram_tensor("pe_y", [M, D], f32, kind="ExternalOutput")
    pey2_dram = nc.dram_tensor("pe_y2", [M, D], f32, kind="ExternalOutput")

    # load groups in pairs-of-slots (pairs 0-1 ride the head wx DMA);
    # small first groups for fast pipeline fill
    sizes = [2, 2, 4]
    while sum(sizes) < C2 - 2:
        sizes.append(min(4, C2 - 2 - sum(sizes)))
    while sum(sizes) > C2 - 2:
        t = sizes.pop()
        excess = sum(sizes) + t - (C2 - 2)
        if t - excess > 0:
            sizes.append(t - excess)
    starts = [2 + sum(sizes[:gi]) for gi in range(len(sizes))]
    g_of_pair = {}
    for gi, w in enumerate(sizes):
        for q in range(w):
            g_of_pair[starts[gi] + q] = gi
    LEADP = 12           # pairs of load lead

    with ExitStack() as ctx:
        tc = ctx.enter_context(tile.TileContext(nc))
        const = ctx.enter_context(tc.tile_pool(name="const", bufs=1))
        # pinned stats banks FIRST -> PSUM banks 0,1
        ps_pin = ctx.enter_context(tc.tile_pool(name="ps_pin", bufs=1, space="PSUM"))
        ps_y = ctx.enter_context(tc.tile_pool(name="ps_y", bufs=2, space="PSUM"))
        xtp = ctx.enter_context(tc.tile_pool(name="xtp", bufs=6))
        ysp = ctx.enter_context(tc.tile_pool(name="ysp", bufs=8))
        stq = ctx.enter_context(tc.tile_pool(name="stq", bufs=1))

        nc.gpsimd.load_library(library_config.attn)

        # prologue tiles: A = [0:600 w_k0 | 600:1624 x0 pairs 0-1]
        #                 B = [0:600 w_k1 | 600:1624 k1 cols 0-1]
        wxa_sb = const.tile([P, 1624], fp8)
        wxb_sb = const.tile([P, 1624], fp8)
        oh_sb = const.tile([P, NPE, M], bf16)
        xt1_sb = const.tile([P, KB, 512], fp8)  # k1: 4 bands at 0/32/64/96

        pinY = ps_pin.tile([P, 512], f32, name="pinY", tag="pinY")
        pinY2 = ps_pin.tile([P, 512], f32, name="pinY2", tag="pinY2")
        nc.vector.memset(pinY[0:M, 0:D], 0.0)
        nc.vector.memset(pinY2[0:M, 0:D], 0.0)

        sall = stq.tile([P, NPOOL, 2 * D], bf16, name="sall", tag="sall")
        stg = stq.tile([P, 2 * D], f32, name="stg", tag="stg")
        nc.sync.dma_start(wxa_sb[:], wxa_dram.ap())
        nc.gpsimd.dma_start(wxb_sb[:], wxb_dram.ap())

        stL = {}          # load group -> xt0 tile
        stY = {}          # triple -> ys tile
        next_load = 0

        def issue_loads(up_to_pair):
            nonlocal next_load
            while next_load < len(sizes) and starts[next_load] <= up_to_pair:
                gi = next_load
                w = sizes[gi]
                q0 = starts[gi]
                xt0 = xtp.tile([P, 4, 512], fp8, name=f"xt0_{gi}", tag="xt0")
                nc.sync.dma_start(
                    xt0[:, 0:w, :],
                    x0_dram.ap()[q0:q0 + w].rearrange("q p c -> p q c"),
                )
                stL[gi] = xt0
                next_load += 1
                if gi == 1:
                    nc.sync.dma_start(oh_sb[:], oh_dram.ap())
                elif gi == 2 and KB > 2:
                    nc.sync.dma_start(xt1_sb[:, 2:KB, :],
                                      x1_dram.ap()[:, 2:KB, :])

        def mains(t):
            w3 = min(3, C - 3 * t)
            y3 = ps_y.tile([P, 1536], f32, name=f"y3_{t}", tag="y3")
            for si in range(w3):
                s = 3 * t + si
                q, sub = s // 2, s % 2
                if q < 2:
                    st_ap = wxa_sb[:, 600 + 512 * q + 256 * sub:
                                   600 + 512 * q + 256 * sub + 256]
                else:
                    gi = g_of_pair[q]
                    xt0 = stL[gi]
                    ql = q - starts[gi]
                    st_ap = xt0[:, ql, 256 * sub:256 * sub + 256]
                nc.tensor.matmul(
                    y3[:, 512 * si:512 * si + D],
                    st_ap.rearrange("p (j i) -> p j i", i=2),
                    wxa_sb[:, 0:600].rearrange("p (k n) -> p k n", n=D),
                    start=True, stop=False, perf_mode=PM.DoubleRowSwInterleave,
                )
            for si in range(w3):
                s = 3 * t + si
                q, sub = s // 2, s % 2
                bo = 32 * (q % 4)
                col = q // 4
                if col < 2:
                    st1_ap = wxb_sb[bo:bo + K1P,
                                    600 + 512 * col + 256 * sub:
                                    600 + 512 * col + 256 * sub + 256]
                else:
                    st1_ap = xt1_sb[bo:bo + K1P, col, 256 * sub:256 * sub + 256]
                nc.tensor.matmul(
                    y3[:, 512 * si:512 * si + D],
                    st1_ap.rearrange("p (j i) -> p j i", i=2),
                    wxb_sb[bo:bo + K1P, 0:600]
                        .rearrange("p (k n) -> p k n", n=D),
                    start=False, stop=True, perf_mode=PM.DoubleRowSwInterleave,
                    tile_position=(bo, 0),
                )
            return y3

        def actdve(t, y3):
            w3 = min(3, C - 3 * t)
            ys = ysp.tile([P, 3, 2 * D], bf16, name=f"ys_{t}", tag="ys")
            nc.scalar.activation(
                ys[:, 0:w3, 0:D],
                y3[:].rearrange("p (s c) -> p s c", s=3)[:, 0:w3, 0:D],
                AF.Gelu,
            )
            nc.vector.tensor_mul(
                ys[:, 0:w3, D:2 * D], ys[:, 0:w3, 0:D], ys[:, 0:w3, 0:D]
            )
            stY[t] = ys

        def stats(t):
            w3 = min(3, C - 3 * t)
            ys = stY.pop(t)
            pe_sis = []
            for si in range(w3):
                s = 3 * t + si
                kind, i = route[s]
                if kind == "pool":
                    nc.gpsimd.partition_all_reduce(
                        sall[:, i, :], ys[:, si, :],
                        channels=P, reduce_op=bass_isa.ReduceOp.add,
                    )
                else:
                    pe_sis.append((si, i))
            for si, i in pe_sis:
                nc.tensor.matmul(
                    pinY[0:M, 0:D], oh_sb[:, i, 0:M], ys[:, si, 0:D],
                    start=False, stop=True, skip_group_check=True,
                )
            for si, i in pe_sis:
                nc.tensor.matmul(
                    pinY2[0:M, 0:D], oh_sb[:, i, 0:M], ys[:, si, D:2 * D],
                    start=False, stop=True, skip_group_check=True,
                )

        LAG = 4           # triples between gelu and its stats (decouples the
                          # PE in-order queue from the ACT->DVE->stats chain)
        y3_of = {}
        for t in range(NT + LAG):
            if t < NT:
                lastpair = min(3 * t + 2 + 2 * LEADP, C - 1) // 2
                issue_loads(lastpair)
                y3_of[t] = mains(t)
            if 1 <= t <= NT:
                actdve(t - 1, y3_of.pop(t - 1))
            if t >= LAG:
                stats(t - LAG)
                if t - LAG == NT - 3:
                    # all Pool ops done (none in the last 2 triples)
                    nc.gpsimd.dma_start(pool_dram.ap(), sall[0:1, :, :])

        # copyY on DVE, copyY2 on the (now idle) ACT engine, in parallel;
        # then two 500ns-floor drains in parallel on SP and ACT
        nc.vector.tensor_copy(stg[0:M, 0:D], pinY[0:M, 0:D])
        nc.scalar.activation(stg[0:M, D:2 * D], pinY2[0:M, 0:D], AF.Copy)
        nc.sync.dma_start(pey_dram.ap(), stg[0:M, 0:D])
        nc.scalar.dma_start(pey2_dram.ap(), stg[0:M, D:2 * D])

    nc.compile()
    return nc


def _pack_inputs(x, W, b, M=16):
    """Host prep. Returns (arrays..., meta) for all cores."""
    f8 = ml_dtypes.float8_e4m3
    bff = ml_dtypes.bfloat16
    x = np.asarray(x, np.float32)
    # padding rows are all -1.0; checking the first 8 dims is exact in
    # practice (P[gaussian row starts with 8 exact -1.0s] ~ 1e-56)
    valid = ~np.all(x[:, :, :8] == -1.0, axis=-1)    # [B, T]
    n = valid.sum(axis=1).astype(np.int64)           # [B]
    total = int(n.sum())

    C = -(-total // (NCORES * P))
    percore = C * P
    padtot = NCORES * percore

    toks = np.zeros((padtot, 302), np.float32)
    toks[:total, :D] = x[valid]
    toks[:total, D] = 1.0       # bias column
    toks[:total, D + 1] = 1.0   # bias fp8-residual column
    btok = np.full(padtot, -1, np.int64)
    btok[:total] = np.repeat(np.arange(B), n)

    pool_idx, pe_idx = _routing(C, M)
    NPOOL, NPE = len(pool_idx), len(pe_idx)

    Cp = C + (C & 1)               # data padded to even slots for pairs
    C2 = Cp // 2
    KB = -(-C2 // 4)
    x0 = np.zeros((NCORES, C2, P, 512), f8)
    x1 = np.zeros((NCORES, P, KB, 512), f8)
    oh = np.zeros((NCORES, P, NPE, M), bff)
    pool_map = np.full((NCORES, NPOOL), -1, np.int64)
    group_map = np.full((NCORES, M), -1, np.int64)

    for c in range(NCORES):
        ct = toks[c * percore:(c + 1) * percore].reshape(C, P, 302)
        cb = btok[c * percore:(c + 1) * percore].reshape(C, P)

        # local batch groups in order of appearance
        gids = {}
        for bid in cb.reshape(-1):
            if bid >= 0 and bid not in gids:
                gids[bid] = len(gids)
        G = len(gids)
        assert G <= M, f"core {c}: {G} batch groups > M={M}"
        for bid, g in gids.items():
            group_map[c, g] = bid

        # slot classification: poolable = at most one real batch in the slot
        nb_per_slot = [np.unique(cb[s][cb[s] >= 0]) for s in range(C)]
        boundary = [s for s in range(C) if len(nb_per_slot[s]) > 1]
        single = [s for s in range(C) if len(nb_per_slot[s]) <= 1]
        assert len(boundary) <= NPE, f"core {c}: too many boundary slots"

        # permutation: old slot -> new index. Boundary slots must land on
        # PE-routed indices.
        old_of_new = np.empty(C, np.int64)
        pe_free = list(pe_idx)
        for s in boundary:
            old_of_new[pe_free.pop(0)] = s
        rest = single
        fill = pool_idx + pe_free
        fill.sort()
        for pos, s in zip(fill, rest):
            old_of_new[pos] = s

        # stationaries (built in old order, then permuted)
        if Cp != C:
            ct = np.concatenate([ct, np.zeros((1, P, 302), np.float32)], 0)
        ctq = ct.astype(f8)
        a = ctq[:, :, 0:256].reshape(Cp, P, 2, 128)
        st0 = np.ascontiguousarray(
            a.transpose(0, 3, 1, 2)[:, :, ::-1, :]).reshape(Cp, P, 256)
        bb = ctq[:, :, 256:302].reshape(Cp, P, K1P, 2)
        st1 = np.ascontiguousarray(
            bb.transpose(0, 2, 1, 3)[:, :, ::-1, :]).reshape(Cp, K1P, 256)
        perm = np.concatenate([old_of_new, np.arange(C, Cp)])
        st0 = st0[perm]
        st1 = st1[perm]
        x0[c] = st0.reshape(C2, 2, P, 256).transpose(0, 2, 1, 3).reshape(
            C2, P, 512)
        # k1 stationaries pair-packed (512B chunks), pair q at partition
        # band 32*(q%4), column q//4 -> all 128 partitions carry payload
        st1p = st1.reshape(C2, 2, K1P, 256).transpose(0, 2, 1, 3).reshape(
            C2, K1P, 512)                                  # [q, p, 512]
        for bq in range(C2):
            x1[c, 32 * (bq % 4):32 * (bq % 4) + K1P, bq // 4, :] = st1p[bq]

        # metadata + one-hots in new order
        for i, pos in enumerate(pool_idx):
            s = old_of_new[pos]
            bs = nb_per_slot[s]
            if len(bs):
                pool_map[c, i] = bs[0]
        for i, pos in enumerate(pe_idx):
            s = old_of_new[pos]
            sb = cb[s]
            for t_ in range(P):
                if sb[t_] >= 0:
                    oh[c, t_, i, gids[sb[t_]]] = 1.0

    wtf = np.zeros((302, D), np.float32)
    wtf[:D, :] = np.asarray(W, np.float32).T
    bf = np.asarray(b, np.float32)
    b8 = bf.astype(f8).astype(np.float32)
    wtf[D, :] = b8                 # fp8-rounded bias
    wtf[D + 1, :] = bf - b8        # residual, cancels bias quantization
    wq = wtf.astype(f8)
    wcomb = np.zeros((P, 4, D), f8)
    wcomb[:, 0:2, :] = wq[0:256].reshape(2, 128, D).transpose(1, 0, 2)
    for b_ in range(4):
        wcomb[32 * b_:32 * b_ + K1P, 2:4, :] = wq[256:302].reshape(K1P, 2, D)
    wxa = np.zeros((NCORES, P, 1624), f8)
    wxa[:, :, 0:600] = wcomb[:, 0:2, :].reshape(1, P, 600)
    wxa[:, :, 600:1624] = x0[:, 0:2].transpose(0, 2, 1, 3).reshape(
        NCORES, P, 1024)
    wxb = np.zeros((NCORES, P, 1624), f8)
    wxb[:, :, 0:600] = wcomb[:, 2:4, :].reshape(1, P, 600)
    wxb[:, :, 600:1624] = x1[:, :, 0:2, :].reshape(NCORES, P, 1024)

    meta = (C, M, NPOOL, pool_map, group_map, n)
    return x0, x1, wxa, wxb, oh, meta


def _epilogue(pool_stats, pe_stats, meta):
    """pool_stats [NC, NPOOL, 600], pe_stats [NC, M, 600] -> out [B, 600]."""
    C, M, NPOOL, pool_map, group_map, n = meta
    acc = np.zeros((B + 1, 2 * D), np.float64)
    np.add.at(acc, np.where(pool_map < 0, B, pool_map).reshape(-1),
              pool_stats.reshape(-1, 2 * D).astype(np.float64))
    np.add.at(acc, np.where(group_map < 0, B, group_map).reshape(-1),
              pe_stats.reshape(-1, 2 * D).astype(np.float64))
    sy = acc[:B, 0:D]
    sy2 = acc[:B, D:2 * D]
    nf = n.astype(np.float64)[:, None]
    with np.errstate(divide="ignore", invalid="ignore"):
        mean = sy / nf
        var = (sy2 - nf * mean * mean) / np.maximum(nf - 1.0, 1.0)
        std = np.where(nf > 1.0, np.sqrt(np.maximum(var, 0.0)), 0.0)
    out = np.concatenate([std, mean], axis=-1)
    out = np.where(np.isnan(out), 0.0, out)
    return out.astype(np.float32)


def _get_nc(C, M, NPOOL):
    key = ("nc", C, M, NPOOL)
    if key not in _cache:
        _cache[key] = _build_nc(C, M, NPOOL)
    return _cache[key]


def _prep(x, W, b):
    for M in (16, 32, 64, 128):
        try:
            x0, x1, wxa, wxb, oh, meta = _pack_inputs(x, W, b, M=M)
            break
        except AssertionError:
            continue
    C, M, NPOOL = meta[0], meta[1], meta[2]
    nc = _get_nc(C, M, NPOOL)
    in_maps = [
        {"x0": x0[c], "x1": x1[c], "wxa": wxa[c], "wxb": wxb[c],
         "oh": np.asarray(oh[c])}
        for c in range(NCORES)
    ]
    return nc, in_maps, meta


def kernel(x, W, b):
    from concourse.bass_utils import run_bass_kernel_spmd

    nc, in_maps, meta = _prep(x, W, b)
    res = run_bass_kernel_spmd(nc, in_maps, core_ids=list(range(NCORES)))
    M = meta[1]
    pool_stats = np.stack([res.results[c]["pool_stats"] for c in range(NCORES)])
    pe_stats = np.stack([
        np.concatenate([res.results[c]["pe_y"], res.results[c]["pe_y2"]],
                       axis=-1) for c in range(NCORES)])
    return _epilogue(pool_stats.astype(np.float64),
                     pe_stats.astype(np.float64), meta)


def sim_prep(x, W, b):
    """Hook for sim_time.py: returns (nc, in_maps); caches meta for sim_check."""
    nc, in_maps, meta = _prep(x, W, b)
    _cache["meta"] = meta
    return nc, in_maps


def sim_check(sim, ins, expected):
    """Hook for sim_time.py: rel err over batches fully on core 0."""
    meta = _cache["meta"]
    C, M, NPOOL, pool_map, group_map, n = meta
    pool_stats = np.zeros((NCORES, NPOOL, 2 * D), np.float64)
    pe_stats = np.zeros((NCORES, M, 2 * D), np.float64)
    pool_stats[0] = np.asarray(sim.tensor("pool_stats")).astype(np.float64)
    pe_stats[0] = np.concatenate(
        [np.asarray(sim.tensor("pe_y")).astype(np.float64),
         np.asarray(sim.tensor("pe_y2")).astype(np.float64)], axis=-1)
    out = _epilogue(pool_stats, pe_stats, meta)
    # batches entirely inside core 0's token window
    bs = sorted(set(int(v) for v in pool_map[0] if v >= 0)
                | set(int(v) for v in group_map[0] if v >= 0))
    others = set(int(v) for v in pool_map[1:].reshape(-1) if v >= 0) \
        | set(int(v) for v in group_map[1:].reshape(-1) if v >= 0)
    bs = [b_ for b_ in bs if b_ not in others]
    return np.abs(out[bs] - expected[bs]).max() / np.abs(expected).max()


# revision 5
# speedup vs baseline: 1.0166x; 1.0147x over previous
"""Trainium2 Bass kernel for nn_LinearNNEncoder (fused Linear+GELU, masked per-batch
mean/std over ragged sequences), data-parallel over 8 NeuronCores.

Contract: kernel(**inputs) takes FULL inputs (x [64,2048,300] f32, W [300,300],
b [300]) and returns the FULL output [64, 600] f32 (concat(std, mean) per batch).

Design (v4):
  - Host drops padding rows and packs the global valid-token stream
    contiguously into 128-token slots, split evenly across 8 cores (batches
    may span slot/core boundaries; the host epilogue re-combines per-batch
    sums). C = ceil(total/1024) slots per core, padded even.
  - Contraction trimmed to K=302 (300 dims + ones/bias + ones/bias-residual):
    fp8 DoubleRowSwInterleave stationaries split as k0 = dims 0..255 on 128
    partitions and k1 = dims 256..301 on 23 partitions; 2 matmuls per slot
    (~62.5ns each, cost scales with the 300-wide output only).
  - GELU on ACT in 3-slot batches from a 3-bank PSUM tile (the wall:
    ~0.833ns/elem); DVE squares the triple in one 2x-mode op.
  - Per-slot stats (sum y, sum y^2 over tokens) routed by slot index:
    ~53% to Pool (one 600-wide partition_all_reduce per slot -> SBUF rows),
    rest to PE (two 125ns one-hot matmuls accumulating start=False into two
    pinned PSUM banks, partitions = local batch group). The per-slot
    [128, M] one-hot is host data, so batch boundaries inside a slot work
    under the SPMD single-program constraint; the host additionally permutes
    slots so boundary slots land on PE-routed indices (Pool needs
    single-batch slots).
  - Drains: Pool rows from SBUF partition 0; pinned banks via 2 DVE copies.
  - Host epilogue (float64): combine sums per batch, torch.std_mean
    semantics (unbiased, n==1 -> std=0), NaN->0.
"""
import numpy as np
import ml_dtypes

B, T, D = 64, 2048, 300
NCORES = 8
P = 128
K1P = 23            # k1 chunk partitions (dims 256..301 as 23x2)

_cache = {}


def _routing(C, M):
    """Fixed per-program routing: which slot indices go to Pool vs PE.
    The last 6 slots are PE-routed so every Pool op (and the pool_stats
    drain) finishes well before the pipeline tail."""
    Ce = C - 6
    npool = int(round(Ce * 0.55))
    pool_idx = []
    pe_idx = []
    acc = 0
    for s in range(C):
        nacc = ((min(s, Ce - 1) + 1) * npool) // Ce if s < Ce else acc
        if nacc > acc:
            pool_idx.append(s)
        else:
            pe_idx.append(s)
        acc = nacc
    return pool_idx, pe_idx


def _build_nc(C, M, NPOOL):
    from contextlib import ExitStack
    import concourse.tile as tile
    from concourse import mybir, bacc, bass_isa, library_config

    f32 = mybir.dt.float32
    bf16 = mybir.dt.bfloat16
    fp8 = mybir.dt.float8e4
    AF = mybir.ActivationFunctionType
    PM = mybir.MatmulPerfMode

    pool_idx, pe_idx = _routing(C, M)
    assert len(pool_idx) == NPOOL
    NPE = C - NPOOL
    route = {}
    for i, s in enumerate(pool_idx):
        route[s] = ("pool", i)
    for i, s in enumerate(pe_idx):
        route[s] = ("pe", i)

    C2 = (C + 1) // 2
    KB = -(-C2 // 4)    # k1 columns: col k holds pairs 4k..4k+3 on 4 bands
    # gelu groups: alternating quad (4-bank PSUM tile) and triple (3-bank),
    # 7 slots per 2 ACT ops; ragged tail group
    GROUPS = []
    s0 = 0
    while s0 < C:
        w = 3 if len(GROUPS) % 2 == 0 else 4
        w = min(w, C - s0)
        GROUPS.append((s0, w))
        s0 += w
    NG = len(GROUPS)

    nc = bacc.Bacc("TRN2", target_bir_lowering=False, debug=False)
    x0_dram = nc.dram_tensor("x0", [C2, P, 512], fp8, kind="ExternalInput")
    x1_dram = nc.dram_tensor("x1", [P, KB, 512], fp8, kind="ExternalInput")
    wxa_dram = nc.dram_tensor("wxa", [P, 1624], fp8, kind="ExternalInput")
    wxb_dram = nc.dram_tensor("wxb", [P, 1624], fp8, kind="ExternalInput")
    oh_dram = nc.dram_tensor("oh", [P, NPE, M], bf16, kind="ExternalInput")
    pool_dram = nc.dram_tensor("pool_stats", [NPOOL, 2 * D], bf16,
                               kind="ExternalOutput")
    pey_dram = nc.dram_tensor("pe_y", [64 + M, D], f32, kind="ExternalOutput")

    # load groups in pairs-of-slots (pairs 0-1 ride the head wx DMA);
    # small first groups for fast pipeline fill
    sizes = [2, 2, 4]
    while sum(sizes) < C2 - 2:
        sizes.append(min(4, C2 - 2 - sum(sizes)))
    while sum(sizes) > C2 - 2:
        t = sizes.pop()
        excess = sum(sizes) + t - (C2 - 2)
        if t - excess > 0:
            sizes.append(t - excess)
    starts = [2 + sum(sizes[:gi]) for gi in range(len(sizes))]
    g_of_pair = {}
    for gi, w in enumerate(sizes):
        for q in range(w):
            g_of_pair[starts[gi] + q] = gi
    LEADP = 12           # pairs of load lead

    with ExitStack() as ctx:
        tc = ctx.enter_context(tile.TileContext(nc))
        const = ctx.enter_context(tc.tile_pool(name="const", bufs=1))
        # pinned stats banks FIRST -> PSUM banks 0,1
        ps_pin = ctx.enter_context(tc.tile_pool(name="ps_pin", bufs=1, space="PSUM"))
        ps_y4 = ctx.enter_context(tc.tile_pool(name="ps_y4", bufs=1, space="PSUM"))
        ps_y3 = ctx.enter_context(tc.tile_pool(name="ps_y3", bufs=1, space="PSUM"))
        xtp = ctx.enter_context(tc.tile_pool(name="xtp", bufs=6))
        ysp = ctx.enter_context(tc.tile_pool(name="ysp", bufs=8))
        stq = ctx.enter_context(tc.tile_pool(name="stq", bufs=1))

        nc.gpsimd.load_library(library_config.attn)

        # prologue tiles: A = [0:600 w_k0 | 600:1624 x0 pairs 0-1]
        #                 B = [0:600 w_k1 | 600:1624 k1 cols 0-1]
        wxa_sb = const.tile([P, 1624], fp8)
        wxb_sb = const.tile([P, 1624], fp8)
        oh_sb = const.tile([P, NPE, M], bf16)
        xt1_sb = const.tile([P, KB, 512], fp8)  # k1: 4 bands at 0/32/64/96

        pin = ps_pin.tile([P, 512], f32, name="pin", tag="pin")
        nc.vector.memset(pin[:, 0:D], 0.0)

        sall = stq.tile([P, NPOOL, 2 * D], bf16, name="sall", tag="sall")
        stgy = stq.tile([P, D], f32, name="stgy", tag="stgy")
        nc.sync.dma_start(wxa_sb[:], wxa_dram.ap())
        nc.gpsimd.dma_start(wxb_sb[:], wxb_dram.ap())

        stL = {}          # load group -> xt0 tile
        stY = {}          # triple -> ys tile
        next_load = 0

        def issue_loads(up_to_pair):
            nonlocal next_load
            while next_load < len(sizes) and starts[next_load] <= up_to_pair:
                gi = next_load
                w = sizes[gi]
                q0 = starts[gi]
                xt0 = xtp.tile([P, 4, 512], fp8, name=f"xt0_{gi}", tag="xt0")
                nc.sync.dma_start(
                    xt0[:, 0:w, :],
                    x0_dram.ap()[q0:q0 + w].rearrange("q p c -> p q c"),
                )
                stL[gi] = xt0
                next_load += 1
                if gi == 1:
                    nc.sync.dma_start(oh_sb[:], oh_dram.ap())
                elif gi == 2 and KB > 2:
                    nc.sync.dma_start(xt1_sb[:, 2:KB, :],
                                      x1_dram.ap()[:, 2:KB, :])

        def mains(t):
            g0, w3 = GROUPS[t]
            if t % 2 == 1:
                y3 = ps_y4.tile([P, 2048], f32, name=f"y4_{t}", tag="y4")
            else:
                y3 = ps_y3.tile([P, 1536], f32, name=f"y3_{t}", tag="y3")
            for si in range(w3):
                s = g0 + si
                q, sub = s // 2, s % 2
                if q < 2:
                    st_ap = wxa_sb[:, 600 + 512 * q + 256 * sub:
                                   600 + 512 * q + 256 * sub + 256]
                else:
                    gi = g_of_pair[q]
                    xt0 = stL[gi]
                    ql = q - starts[gi]
                    st_ap = xt0[:, ql, 256 * sub:256 * sub + 256]
                nc.tensor.matmul(
                    y3[:, 512 * si:512 * si + D],
                    st_ap.rearrange("p (j i) -> p j i", i=2),
                    wxa_sb[:, 0:600].rearrange("p (k n) -> p k n", n=D),
                    start=True, stop=False, perf_mode=PM.DoubleRowSwInterleave,
                )
            for si in range(w3):
                s = g0 + si
                q, sub = s // 2, s % 2
                bo = 32 * (q % 4)
                col = q // 4
                if col < 2:
                    st1_ap = wxb_sb[bo:bo + K1P,
                                    600 + 512 * col + 256 * sub:
                                    600 + 512 * col + 256 * sub + 256]
                else:
                    st1_ap = xt1_sb[bo:bo + K1P, col, 256 * sub:256 * sub + 256]
                nc.tensor.matmul(
                    y3[:, 512 * si:512 * si + D],
                    st1_ap.rearrange("p (j i) -> p j i", i=2),
                    wxb_sb[bo:bo + K1P, 0:600]
                        .rearrange("p (k n) -> p k n", n=D),
                    start=False, stop=True, perf_mode=PM.DoubleRowSwInterleave,
                    tile_position=(bo, 0),
                )
            return y3

        def actdve(t, y3):
            g0, w3 = GROUPS[t]
            nb = 4 if t % 2 == 1 else 3
            ys = ysp.tile([P, 4, 2 * D], bf16, name=f"ys_{t}", tag="ys")
            nc.scalar.activation(
                ys[:, 0:w3, 0:D],
                y3[:].rearrange("p (s c) -> p s c", s=nb)[:, 0:w3, 0:D],
                AF.Gelu,
            )
            nc.vector.tensor_mul(
                ys[:, 0:w3, D:2 * D], ys[:, 0:w3, 0:D], ys[:, 0:w3, 0:D]
            )
            stY[t] = ys

        def stats(t):
            g0, w3 = GROUPS[t]
            ys = stY.pop(t)
            pe_sis = []
            for si in range(w3):
                s = g0 + si
                kind, i = route[s]
                if kind == "pool":
                    nc.gpsimd.partition_all_reduce(
                        sall[:, i, :], ys[:, si, :],
                        channels=P, reduce_op=bass_isa.ReduceOp.add,
                    )
                else:
                    pe_sis.append((si, i))
            for si, i in pe_sis:
                nc.tensor.matmul(
                    pin[0:M, 0:D], oh_sb[:, i, 0:M], ys[:, si, 0:D],
                    start=False, stop=True, skip_group_check=True,
                )
            for si, i in pe_sis:
                nc.tensor.matmul(
                    pin[64:64 + M, 0:D], oh_sb[:, i, 0:M], ys[:, si, D:2 * D],
                    start=False, stop=True, skip_group_check=True,
                )

        LAG = 4           # triples between gelu and its stats (decouples the
                          # PE in-order queue from the ACT->DVE->stats chain)
        y3_of = {}
        for t in range(NG + LAG):
            if t < NG:
                lastpair = min(GROUPS[t][0] + GROUPS[t][1] - 1
                               + 2 * LEADP, C - 1) // 2
                issue_loads(lastpair)
                y3_of[t] = mains(t)
            if 1 <= t <= NG:
                actdve(t - 1, y3_of.pop(t - 1))
            if t >= LAG:
                stats(t - LAG)
                if GROUPS[t - LAG][0] + GROUPS[t - LAG][1] >= C - 6:
                    if not stL.get("pooldrained"):
                        stL["pooldrained"] = True
                        nc.gpsimd.dma_start(pool_dram.ap(), sall[0:1, :, :])

        # copyY on DVE, copyY2 on the (now idle) ACT engine, in parallel;
        # then two 500ns-floor drains in parallel on SP and ACT
        # one copy of partitions 0:80 (16:64 are memset junk) and one drain
        nc.scalar.activation(stgy[0:64 + M, 0:D], pin[0:64 + M, 0:D], AF.Copy)
        nc.scalar.dma_start(pey_dram.ap(), stgy[0:64 + M, 0:D])

    nc.compile()
    return nc


def _pack_inputs(x, W, b, M=16):
    """Host prep. Returns (arrays..., meta) for all cores."""
    f8 = ml_dtypes.float8_e4m3
    bff = ml_dtypes.bfloat16
    x = np.asarray(x, np.float32)
    # padding rows are all -1.0; checking the first 8 dims is exact in
    # practice (P[gaussian row starts with 8 exact -1.0s] ~ 1e-56)
    valid = ~np.all(x[:, :, :8] == -1.0, axis=-1)    # [B, T]
    n = valid.sum(axis=1).astype(np.int64)           # [B]
    total = int(n.sum())

    C = -(-total // (NCORES * P))
    percore = C * P
    padtot = NCORES * percore

    toks = np.zeros((padtot, 302), np.float32)
    toks[:total, :D] = x[valid]
    toks[:total, D] = 1.0       # bias column
    toks[:total, D + 1] = 1.0   # bias fp8-residual column
    btok = np.full(padtot, -1, np.int64)
    btok[:total] = np.repeat(np.arange(B), n)

    pool_idx, pe_idx = _routing(C, M)
    NPOOL, NPE = len(pool_idx), len(pe_idx)

    Cp = C + (C & 1)               # data padded to even slots for pairs
    C2 = Cp // 2
    KB = -(-C2 // 4)
    x0 = np.zeros((NCORES, C2, P, 512), f8)
    x1 = np.zeros((NCORES, P, KB, 512), f8)
    oh = np.zeros((NCORES, P, NPE, M), bff)
    pool_map = np.full((NCORES, NPOOL), -1, np.int64)
    group_map = np.full((NCORES, M), -1, np.int64)

    for c in range(NCORES):
        ct = toks[c * percore:(c + 1) * percore].reshape(C, P, 302)
        cb = btok[c * percore:(c + 1) * percore].reshape(C, P)

        # local batch groups in order of appearance
        gids = {}
        for bid in cb.reshape(-1):
            if bid >= 0 and bid not in gids:
                gids[bid] = len(gids)
        G = len(gids)
        assert G <= M, f"core {c}: {G} batch groups > M={M}"
        for bid, g in gids.items():
            group_map[c, g] = bid

        # slot classification: poolable = at most one real batch in the slot
        nb_per_slot = [np.unique(cb[s][cb[s] >= 0]) for s in range(C)]
        boundary = [s for s in range(C) if len(nb_per_slot[s]) > 1]
        single = [s for s in range(C) if len(nb_per_slot[s]) <= 1]
        assert len(boundary) <= NPE, f"core {c}: too many boundary slots"

        # permutation: old slot -> new index. Boundary slots must land on
        # PE-routed indices.
        old_of_new = np.empty(C, np.int64)
        pe_free = list(pe_idx)
        for s in boundary:
            old_of_new[pe_free.pop(0)] = s
        rest = single
        fill = pool_idx + pe_free
        fill.sort()
        for pos, s in zip(fill, rest):
            old_of_new[pos] = s

        # stationaries (built in old order, then permuted)
        if Cp != C:
            ct = np.concatenate([ct, np.zeros((1, P, 302), np.float32)], 0)
        ctq = ct.astype(f8)
        a = ctq[:, :, 0:256].reshape(Cp, P, 2, 128)
        st0 = np.ascontiguousarray(
            a.transpose(0, 3, 1, 2)[:, :, ::-1, :]).reshape(Cp, P, 256)
        bb = ctq[:, :, 256:302].reshape(Cp, P, K1P, 2)
        st1 = np.ascontiguousarray(
            bb.transpose(0, 2, 1, 3)[:, :, ::-1, :]).reshape(Cp, K1P, 256)
        perm = np.concatenate([old_of_new, np.arange(C, Cp)])
        st0 = st0[perm]
        st1 = st1[perm]
        x0[c] = st0.reshape(C2, 2, P, 256).transpose(0, 2, 1, 3).reshape(
            C2, P, 512)
        # k1 stationaries pair-packed (512B chunks), pair q at partition
        # band 32*(q%4), column q//4 -> all 128 partitions carry payload
        st1p = st1.reshape(C2, 2, K1P, 256).transpose(0, 2, 1, 3).reshape(
            C2, K1P, 512)                                  # [q, p, 512]
        for bq in range(C2):
            x1[c, 32 * (bq % 4):32 * (bq % 4) + K1P, bq // 4, :] = st1p[bq]

        # metadata + one-hots in new order
        for i, pos in enumerate(pool_idx):
            s = old_of_new[pos]
            bs = nb_per_slot[s]
            if len(bs):
                pool_map[c, i] = bs[0]
        for i, pos in enumerate(pe_idx):
            s = old_of_new[pos]
            sb = cb[s]
            for t_ in range(P):
                if sb[t_] >= 0:
                    oh[c, t_, i, gids[sb[t_]]] = 1.0

    wtf = np.zeros((302, D), np.float32)
    wtf[:D, :] = np.asarray(W, np.float32).T
    bf = np.asarray(b, np.float32)
    b8 = bf.astype(f8).astype(np.float32)
    wtf[D, :] = b8                 # fp8-rounded bias
    wtf[D + 1, :] = bf - b8        # residual, cancels bias quantization
    wq = wtf.astype(f8)
    wcomb = np.zeros((P, 4, D), f8)
    wcomb[:, 0:2, :] = wq[0:256].reshape(2, 128, D).transpose(1, 0, 2)
    for b_ in range(4):
        wcomb[32 * b_:32 * b_ + K1P, 2:4, :] = wq[256:302].reshape(K1P, 2, D)
    wxa = np.zeros((NCORES, P, 1624), f8)
    wxa[:, :, 0:600] = wcomb[:, 0:2, :].reshape(1, P, 600)
    wxa[:, :, 600:1624] = x0[:, 0:2].transpose(0, 2, 1, 3).reshape(
        NCORES, P, 1024)
    wxb = np.zeros((NCORES, P, 1624), f8)
    wxb[:, :, 0:600] = wcomb[:, 2:4, :].reshape(1, P, 600)
    wxb[:, :, 600:1624] = x1[:, :, 0:2, :].reshape(NCORES, P, 1024)

    meta = (C, M, NPOOL, pool_map, group_map, n)
    return x0, x1, wxa, wxb, oh, meta


def _epilogue(pool_stats, pe_stats, meta):
    """pool_stats [NC, NPOOL, 600], pe_stats [NC, M, 600] -> out [B, 600]."""
    C, M, NPOOL, pool_map, group_map, n = meta
    acc = np.zeros((B + 1, 2 * D), np.float64)
    np.add.at(acc, np.where(pool_map < 0, B, pool_map).reshape(-1),
              pool_stats.reshape(-1, 2 * D).astype(np.float64))
    np.add.at(acc, np.where(group_map < 0, B, group_map).reshape(-1),
              pe_stats.reshape(-1, 2 * D).astype(np.float64))
    sy = acc[:B, 0:D]
    sy2 = acc[:B, D:2 * D]
    nf = n.astype(np.float64)[:, None]
    with np.errstate(divide="ignore", invalid="ignore"):
        mean = sy / nf
        var = (sy2 - nf * mean * mean) / np.maximum(nf - 1.0, 1.0)
        std = np.where(nf > 1.0, np.sqrt(np.maximum(var, 0.0)), 0.0)
    out = np.concatenate([std, mean], axis=-1)
    out = np.where(np.isnan(out), 0.0, out)
    return out.astype(np.float32)


def _get_nc(C, M, NPOOL):
    key = ("nc", C, M, NPOOL)
    if key not in _cache:
        _cache[key] = _build_nc(C, M, NPOOL)
    return _cache[key]


def _prep(x, W, b):
    for M in (16, 32, 64, 128):
        try:
            x0, x1, wxa, wxb, oh, meta = _pack_inputs(x, W, b, M=M)
            break
        except AssertionError:
            continue
    C, M, NPOOL = meta[0], meta[1], meta[2]
    nc = _get_nc(C, M, NPOOL)
    in_maps = [
        {"x0": x0[c], "x1": x1[c], "wxa": wxa[c], "wxb": wxb[c],
         "oh": np.asarray(oh[c])}
        for c in range(NCORES)
    ]
    return nc, in_maps, meta


def kernel(x, W, b):
    from concourse.bass_utils import run_bass_kernel_spmd

    nc, in_maps, meta = _prep(x, W, b)
    res = run_bass_kernel_spmd(nc, in_maps, core_ids=list(range(NCORES)))
    M = meta[1]
    pool_stats = np.stack([res.results[c]["pool_stats"] for c in range(NCORES)])
    pe_stats = np.stack([
        np.concatenate([res.results[c]["pe_y"][0:M],
                        res.results[c]["pe_y"][64:64 + M]], axis=-1)
        for c in range(NCORES)])
    return _epilogue(pool_stats.astype(np.float64),
                     pe_stats.astype(np.float64), meta)


def sim_prep(x, W, b):
    """Hook for sim_time.py: returns (nc, in_maps); caches meta for sim_check."""
    nc, in_maps, meta = _prep(x, W, b)
    _cache["meta"] = meta
    return nc, in_maps


def sim_check(sim, ins, expected):
    """Hook for sim_time.py: rel err over batches fully on core 0."""
    meta = _cache["meta"]
    C, M, NPOOL, pool_map, group_map, n = meta
    pool_stats = np.zeros((NCORES, NPOOL, 2 * D), np.float64)
    pe_stats = np.zeros((NCORES, M, 2 * D), np.float64)
    pool_stats[0] = np.asarray(sim.tensor("pool_stats")).astype(np.float64)
    _pe = np.asarray(sim.tensor("pe_y")).astype(np.float64)
    pe_stats[0] = np.concatenate([_pe[0:M], _pe[64:64 + M]], axis=-1)
    out = _epilogue(pool_stats, pe_stats, meta)
    # batches entirely inside core 0's token window
    bs = sorted(set(int(v) for v in pool_map[0] if v >= 0)
                | set(int(v) for v in group_map[0] if v >= 0))
    others = set(int(v) for v in pool_map[1:].reshape(-1) if v >= 0) \
        | set(int(v) for v in group_map[1:].reshape(-1) if v >= 0)
    bs = [b_ for b_ in bs if b_ not in others]
    return np.abs(out[bs] - expected[bs]).max() / np.abs(expected).max()


# revision 6
# speedup vs baseline: 1.0259x; 1.0091x over previous
"""Trainium2 Bass kernel for nn_LinearNNEncoder (fused Linear+GELU, masked per-batch
mean/std over ragged sequences), data-parallel over 8 NeuronCores.

Contract: kernel(**inputs) takes FULL inputs (x [64,2048,300] f32, W [300,300],
b [300]) and returns the FULL output [64, 600] f32 (concat(std, mean) per batch).

Design (v4):
  - Host drops padding rows and packs the global valid-token stream
    contiguously into 128-token slots, split evenly across 8 cores (batches
    may span slot/core boundaries; the host epilogue re-combines per-batch
    sums). C = ceil(total/1024) slots per core, padded even.
  - Contraction trimmed to K=302 (300 dims + ones/bias + ones/bias-residual):
    fp8 DoubleRowSwInterleave stationaries split as k0 = dims 0..255 on 128
    partitions and k1 = dims 256..301 on 23 partitions; 2 matmuls per slot
    (~62.5ns each, cost scales with the 300-wide output only).
  - GELU on ACT in 3-slot batches from a 3-bank PSUM tile (the wall:
    ~0.833ns/elem); DVE squares the triple in one 2x-mode op.
  - Per-slot stats (sum y, sum y^2 over tokens) routed by slot index:
    ~53% to Pool (one 600-wide partition_all_reduce per slot -> SBUF rows),
    rest to PE (two 125ns one-hot matmuls accumulating start=False into two
    pinned PSUM banks, partitions = local batch group). The per-slot
    [128, M] one-hot is host data, so batch boundaries inside a slot work
    under the SPMD single-program constraint; the host additionally permutes
    slots so boundary slots land on PE-routed indices (Pool needs
    single-batch slots).
  - Drains: Pool rows from SBUF partition 0; pinned banks via 2 DVE copies.
  - Host epilogue (float64): combine sums per batch, torch.std_mean
    semantics (unbiased, n==1 -> std=0), NaN->0.
"""
import numpy as np
import ml_dtypes

B, T, D = 64, 2048, 300
NCORES = 8
P = 128
K1P = 23            # k1 chunk partitions (dims 256..301 as 23x2)

_cache = {}


def _routing(C, M):
    """Fixed per-program routing: which slot indices go to Pool vs PE.
    The last 6 slots are PE-routed so every Pool op (and the pool_stats
    drain) finishes well before the pipeline tail."""
    Ce = C - 6
    npool = int(round(Ce * 0.55))
    pool_idx = []
    pe_idx = []
    acc = 0
    for s in range(C):
        nacc = ((min(s, Ce - 1) + 1) * npool) // Ce if s < Ce else acc
        if nacc > acc:
            pool_idx.append(s)
        else:
            pe_idx.append(s)
        acc = nacc
    return pool_idx, pe_idx


def _build_nc(C, M, NPOOL):
    from contextlib import ExitStack
    import concourse.tile as tile
    from concourse import mybir, bacc, bass_isa, library_config

    f32 = mybir.dt.float32
    bf16 = mybir.dt.bfloat16
    fp8 = mybir.dt.float8e4
    AF = mybir.ActivationFunctionType
    PM = mybir.MatmulPerfMode

    pool_idx, pe_idx = _routing(C, M)
    assert len(pool_idx) == NPOOL
    NPE = C - NPOOL
    route = {}
    for i, s in enumerate(pool_idx):
        route[s] = ("pool", i)
    for i, s in enumerate(pe_idx):
        route[s] = ("pe", i)

    C2 = (C + 1) // 2
    KB = -(-C2 // 4)    # k1 columns: col k holds pairs 4k..4k+3 on 4 bands
    # gelu groups: alternating quad (4-bank PSUM tile) and triple (3-bank),
    # 7 slots per 2 ACT ops; ragged tail group
    GROUPS = []
    s0 = 0
    while s0 < C:
        w = 3 if len(GROUPS) % 2 == 0 else 4
        w = min(w, C - s0)
        GROUPS.append((s0, w))
        s0 += w
    NG = len(GROUPS)

    nc = bacc.Bacc("TRN2", target_bir_lowering=False, debug=False)
    x0_dram = nc.dram_tensor("x0", [C2, P, 512], fp8, kind="ExternalInput")
    x1_dram = nc.dram_tensor("x1", [P, KB, 512], fp8, kind="ExternalInput")
    wxa_dram = nc.dram_tensor("wxa", [P, 1624], fp8, kind="ExternalInput")
    wxb_dram = nc.dram_tensor("wxb", [P, 1624], fp8, kind="ExternalInput")
    oh_dram = nc.dram_tensor("oh", [P, NPE, M], bf16, kind="ExternalInput")
    pool_dram = nc.dram_tensor("pool_stats", [NPOOL, 2 * D], bf16,
                               kind="ExternalOutput")
    pey_dram = nc.dram_tensor("pe_y", [64 + M, D], f32, kind="ExternalOutput")

    # load groups in pairs-of-slots (pairs 0-1 ride the head wx DMA);
    # small first groups for fast pipeline fill
    sizes = [2, 2, 4]
    while sum(sizes) < C2 - 2:
        sizes.append(min(4, C2 - 2 - sum(sizes)))
    while sum(sizes) > C2 - 2:
        t = sizes.pop()
        excess = sum(sizes) + t - (C2 - 2)
        if t - excess > 0:
            sizes.append(t - excess)
    starts = [2 + sum(sizes[:gi]) for gi in range(len(sizes))]
    g_of_pair = {}
    for gi, w in enumerate(sizes):
        for q in range(w):
            g_of_pair[starts[gi] + q] = gi
    LEADP = 12           # pairs of load lead

    with ExitStack() as ctx:
        tc = ctx.enter_context(tile.TileContext(nc))
        const = ctx.enter_context(tc.tile_pool(name="const", bufs=1))
        # pinned stats banks FIRST -> PSUM banks 0,1
        ps_pin = ctx.enter_context(tc.tile_pool(name="ps_pin", bufs=1, space="PSUM"))
        ps_y4 = ctx.enter_context(tc.tile_pool(name="ps_y4", bufs=1, space="PSUM"))
        ps_y3 = ctx.enter_context(tc.tile_pool(name="ps_y3", bufs=1, space="PSUM"))
        xtp = ctx.enter_context(tc.tile_pool(name="xtp", bufs=6))
        ysp = ctx.enter_context(tc.tile_pool(name="ysp", bufs=8))
        stq = ctx.enter_context(tc.tile_pool(name="stq", bufs=1))

        nc.gpsimd.load_library(library_config.attn)

        # prologue tiles: A = [0:600 w_k0 | 600:1624 x0 pairs 0-1]
        #                 B = [0:600 w_k1 | 600:1624 k1 cols 0-1]
        wxa_sb = const.tile([P, 1624], fp8)
        wxb_sb = const.tile([P, 1624], fp8)
        oh_sb = const.tile([P, NPE, M], bf16)
        xt1_sb = const.tile([P, KB, 512], fp8)  # k1: 4 bands at 0/32/64/96

        pin = ps_pin.tile([P, 512], f32, name="pin", tag="pin")
        nc.vector.memset(pin[:, 0:D], 0.0)

        sall = stq.tile([P, NPOOL, 2 * D], bf16, name="sall", tag="sall")
        stgy = stq.tile([P, D], f32, name="stgy", tag="stgy")
        nc.sync.dma_start(wxa_sb[:], wxa_dram.ap())
        nc.gpsimd.dma_start(wxb_sb[:], wxb_dram.ap())

        stL = {}          # load group -> xt0 tile
        stY = {}          # triple -> ys tile
        next_load = 0

        def issue_loads(up_to_pair):
            nonlocal next_load
            while next_load < len(sizes) and starts[next_load] <= up_to_pair:
                gi = next_load
                w = sizes[gi]
                q0 = starts[gi]
                xt0 = xtp.tile([P, 4, 512], fp8, name=f"xt0_{gi}", tag="xt0")
                nc.sync.dma_start(
                    xt0[:, 0:w, :],
                    x0_dram.ap()[q0:q0 + w].rearrange("q p c -> p q c"),
                )
                stL[gi] = xt0
                next_load += 1
                if gi == 1:
                    nc.sync.dma_start(oh_sb[:], oh_dram.ap())
                elif gi == 2 and KB > 2:
                    nc.sync.dma_start(xt1_sb[:, 2:KB, :],
                                      x1_dram.ap()[:, 2:KB, :])

        def mains(t):
            g0, w3 = GROUPS[t]
            if t % 2 == 1:
                y3 = ps_y4.tile([P, 2048], f32, name=f"y4_{t}", tag="y4")
            else:
                y3 = ps_y3.tile([P, 1536], f32, name=f"y3_{t}", tag="y3")
            for si in range(w3):
                s = g0 + si
                q, sub = s // 2, s % 2
                if q < 2:
                    st_ap = wxa_sb[:, 600 + 512 * q + 256 * sub:
                                   600 + 512 * q + 256 * sub + 256]
                else:
                    gi = g_of_pair[q]
                    xt0 = stL[gi]
                    ql = q - starts[gi]
                    st_ap = xt0[:, ql, 256 * sub:256 * sub + 256]
                nc.tensor.matmul(
                    y3[:, 512 * si:512 * si + D],
                    st_ap.rearrange("p (j i) -> p j i", i=2),
                    wxa_sb[:, 0:600].rearrange("p (k n) -> p k n", n=D),
                    start=True, stop=False, perf_mode=PM.DoubleRowSwInterleave,
                )
            for si in range(w3):
                s = g0 + si
                q, sub = s // 2, s % 2
                bo = 32 * (q % 4)
                col = q // 4
                if col < 2:
                    st1_ap = wxb_sb[bo:bo + K1P,
                                    600 + 512 * col + 256 * sub:
                                    600 + 512 * col + 256 * sub + 256]
                else:
                    st1_ap = xt1_sb[bo:bo + K1P, col, 256 * sub:256 * sub + 256]
                nc.tensor.matmul(
                    y3[:, 512 * si:512 * si + D],
                    st1_ap.rearrange("p (j i) -> p j i", i=2),
                    wxb_sb[bo:bo + K1P, 0:600]
                        .rearrange("p (k n) -> p k n", n=D),
                    start=False, stop=True, perf_mode=PM.DoubleRowSwInterleave,
                    tile_position=(bo, 0),
                )
            return y3

        def actdve(t, y3):
            g0, w3 = GROUPS[t]
            nb = 4 if t % 2 == 1 else 3
            ys = ysp.tile([P, 4, 2 * D], bf16, name=f"ys_{t}", tag="ys")
            nc.scalar.activation(
                ys[:, 0:w3, 0:D],
                y3[:].rearrange("p (s c) -> p s c", s=nb)[:, 0:w3, 0:D],
                AF.Gelu,
            )
            if t == NG - 2 and w3 == 4:
                # final quad: split the square so its stats unblock earlier
                nc.vector.tensor_mul(
                    ys[:, 0:2, D:2 * D], ys[:, 0:2, 0:D], ys[:, 0:2, 0:D]
                )
                nc.vector.tensor_mul(
                    ys[:, 2:4, D:2 * D], ys[:, 2:4, 0:D], ys[:, 2:4, 0:D]
                )
            else:
                nc.vector.tensor_mul(
                    ys[:, 0:w3, D:2 * D], ys[:, 0:w3, 0:D], ys[:, 0:w3, 0:D]
                )
            stY[t] = ys

        def stats(t):
            g0, w3 = GROUPS[t]
            ys = stY.pop(t)
            pe_sis = []
            for si in range(w3):
                s = g0 + si
                kind, i = route[s]
                if kind == "pool":
                    nc.gpsimd.partition_all_reduce(
                        sall[:, i, :], ys[:, si, :],
                        channels=P, reduce_op=bass_isa.ReduceOp.add,
                    )
                else:
                    pe_sis.append((si, i))
            for si, i in pe_sis:
                nc.tensor.matmul(
                    pin[0:M, 0:D], oh_sb[:, i, 0:M], ys[:, si, 0:D],
                    start=False, stop=True, skip_group_check=True,
                )
            for si, i in pe_sis:
                nc.tensor.matmul(
                    pin[64:64 + M, 0:D], oh_sb[:, i, 0:M], ys[:, si, D:2 * D],
                    start=False, stop=True, skip_group_check=True,
                )

        LAG = 4           # triples between gelu and its stats (decouples the
                          # PE in-order queue from the ACT->DVE->stats chain)
        y3_of = {}
        for t in range(NG + LAG):
            if t < NG:
                lastpair = min(GROUPS[t][0] + GROUPS[t][1] - 1
                               + 2 * LEADP, C - 1) // 2
                issue_loads(lastpair)
                y3_of[t] = mains(t)
            if 1 <= t <= NG:
                actdve(t - 1, y3_of.pop(t - 1))
            if t >= LAG:
                stats(t - LAG)
                if GROUPS[t - LAG][0] + GROUPS[t - LAG][1] >= C - 6:
                    if not stL.get("pooldrained"):
                        stL["pooldrained"] = True
                        nc.gpsimd.dma_start(pool_dram.ap(), sall[0:1, :, :])

        # copyY on DVE, copyY2 on the (now idle) ACT engine, in parallel;
        # then two 500ns-floor drains in parallel on SP and ACT
        # one copy of partitions 0:80 (16:64 are memset junk) and one drain
        nc.scalar.activation(stgy[0:64 + M, 0:D], pin[0:64 + M, 0:D], AF.Copy)
        nc.scalar.dma_start(pey_dram.ap(), stgy[0:64 + M, 0:D])

    nc.compile()
    return nc


def _pack_inputs(x, W, b, M=16):
    """Host prep. Returns (arrays..., meta) for all cores."""
    f8 = ml_dtypes.float8_e4m3
    bff = ml_dtypes.bfloat16
    x = np.asarray(x, np.float32)
    # padding rows are all -1.0; checking the first 8 dims is exact in
    # practice (P[gaussian row starts with 8 exact -1.0s] ~ 1e-56)
    valid = ~np.all(x[:, :, :8] == -1.0, axis=-1)    # [B, T]
    n = valid.sum(axis=1).astype(np.int64)           # [B]
    total = int(n.sum())

    C = -(-total // (NCORES * P))
    percore = C * P
    padtot = NCORES * percore

    toks = np.zeros((padtot, 302), np.float32)
    toks[:total, :D] = x[valid]
    toks[:total, D] = 1.0       # bias column
    toks[:total, D + 1] = 1.0   # bias fp8-residual column
    btok = np.full(padtot, -1, np.int64)
    btok[:total] = np.repeat(np.arange(B), n)

    pool_idx, pe_idx = _routing(C, M)
    NPOOL, NPE = len(pool_idx), len(pe_idx)

    Cp = C + (C & 1)               # data padded to even slots for pairs
    C2 = Cp // 2
    KB = -(-C2 // 4)
    x0 = np.zeros((NCORES, C2, P, 512), f8)
    x1 = np.zeros((NCORES, P, KB, 512), f8)
    oh = np.zeros((NCORES, P, NPE, M), bff)
    pool_map = np.full((NCORES, NPOOL), -1, np.int64)
    group_map = np.full((NCORES, M), -1, np.int64)

    for c in range(NCORES):
        ct = toks[c * percore:(c + 1) * percore].reshape(C, P, 302)
        cb = btok[c * percore:(c + 1) * percore].reshape(C, P)

        # local batch groups in order of appearance
        gids = {}
        for bid in cb.reshape(-1):
            if bid >= 0 and bid not in gids:
                gids[bid] = len(gids)
        G = len(gids)
        assert G <= M, f"core {c}: {G} batch groups > M={M}"
        for bid, g in gids.items():
            group_map[c, g] = bid

        # slot classification: poolable = at most one real batch in the slot
        nb_per_slot = [np.unique(cb[s][cb[s] >= 0]) for s in range(C)]
        boundary = [s for s in range(C) if len(nb_per_slot[s]) > 1]
        single = [s for s in range(C) if len(nb_per_slot[s]) <= 1]
        assert len(boundary) <= NPE, f"core {c}: too many boundary slots"

        # permutation: old slot -> new index. Boundary slots must land on
        # PE-routed indices.
        old_of_new = np.empty(C, np.int64)
        pe_free = list(pe_idx)
        for s in boundary:
            old_of_new[pe_free.pop(0)] = s
        rest = single
        fill = pool_idx + pe_free
        fill.sort()
        for pos, s in zip(fill, rest):
            old_of_new[pos] = s

        # stationaries (built in old order, then permuted)
        if Cp != C:
            ct = np.concatenate([ct, np.zeros((1, P, 302), np.float32)], 0)
        ctq = ct.astype(f8)
        a = ctq[:, :, 0:256].reshape(Cp, P, 2, 128)
        st0 = np.ascontiguousarray(
            a.transpose(0, 3, 1, 2)[:, :, ::-1, :]).reshape(Cp, P, 256)
        bb = ctq[:, :, 256:302].reshape(Cp, P, K1P, 2)
        st1 = np.ascontiguousarray(
            bb.transpose(0, 2, 1, 3)[:, :, ::-1, :]).reshape(Cp, K1P, 256)
        perm = np.concatenate([old_of_new, np.arange(C, Cp)])
        st0 = st0[perm]
        st1 = st1[perm]
        x0[c] = st0.reshape(C2, 2, P, 256).transpose(0, 2, 1, 3).reshape(
            C2, P, 512)
        # k1 stationaries pair-packed (512B chunks), pair q at partition
        # band 32*(q%4), column q//4 -> all 128 partitions carry payload
        st1p = st1.reshape(C2, 2, K1P, 256).transpose(0, 2, 1, 3).reshape(
            C2, K1P, 512)                                  # [q, p, 512]
        for bq in range(C2):
            x1[c, 32 * (bq % 4):32 * (bq % 4) + K1P, bq // 4, :] = st1p[bq]

        # metadata + one-hots in new order
        for i, pos in enumerate(pool_idx):
            s = old_of_new[pos]
            bs = nb_per_slot[s]
            if len(bs):
                pool_map[c, i] = bs[0]
        for i, pos in enumerate(pe_idx):
            s = old_of_new[pos]
            sb = cb[s]
            for t_ in range(P):
                if sb[t_] >= 0:
                    oh[c, t_, i, gids[sb[t_]]] = 1.0

    wtf = np.zeros((302, D), np.float32)
    wtf[:D, :] = np.asarray(W, np.float32).T
    bf = np.asarray(b, np.float32)
    b8 = bf.astype(f8).astype(np.float32)
    wtf[D, :] = b8                 # fp8-rounded bias
    wtf[D + 1, :] = bf - b8        # residual, cancels bias quantization
    wq = wtf.astype(f8)
    wcomb = np.zeros((P, 4, D), f8)
    wcomb[:, 0:2, :] = wq[0:256].reshape(2, 128, D).transpose(1, 0, 2)
    for b_ in range(4):
        wcomb[32 * b_:32 * b_ + K1P, 2:4, :] = wq[256:302].reshape(K1P, 2, D)
    wxa = np.zeros((NCORES, P, 1624), f8)
    wxa[:, :, 0:600] = wcomb[:, 0:2, :].reshape(1, P, 600)
    wxa[:, :, 600:1624] = x0[:, 0:2].transpose(0, 2, 1, 3).reshape(
        NCORES, P, 1024)
    wxb = np.zeros((NCORES, P, 1624), f8)
    wxb[:, :, 0:600] = wcomb[:, 2:4, :].reshape(1, P, 600)
    wxb[:, :, 600:1624] = x1[:, :, 0:2, :].reshape(NCORES, P, 1024)

    meta = (C, M, NPOOL, pool_map, group_map, n)
    return x0, x1, wxa, wxb, oh, meta


def _epilogue(pool_stats, pe_stats, meta):
    """pool_stats [NC, NPOOL, 600], pe_stats [NC, M, 600] -> out [B, 600]."""
    C, M, NPOOL, pool_map, group_map, n = meta
    acc = np.zeros((B + 1, 2 * D), np.float64)
    np.add.at(acc, np.where(pool_map < 0, B, pool_map).reshape(-1),
              pool_stats.reshape(-1, 2 * D).astype(np.float64))
    np.add.at(acc, np.where(group_map < 0, B, group_map).reshape(-1),
              pe_stats.reshape(-1, 2 * D).astype(np.float64))
    sy = acc[:B, 0:D]
    sy2 = acc[:B, D:2 * D]
    nf = n.astype(np.float64)[:, None]
    with np.errstate(divide="ignore", invalid="ignore"):
        mean = sy / nf
        var = (sy2 - nf * mean * mean) / np.maximum(nf - 1.0, 1.0)
        std = np.where(nf > 1.0, np.sqrt(np.maximum(var, 0.0)), 0.0)
    out = np.concatenate([std, mean], axis=-1)
    out = np.where(np.isnan(out), 0.0, out)
    return out.astype(np.float32)


def _get_nc(C, M, NPOOL):
    key = ("nc", C, M, NPOOL)
    if key not in _cache:
        _cache[key] = _build_nc(C, M, NPOOL)
    return _cache[key]


def _prep(x, W, b):
    for M in (16, 32, 64, 128):
        try:
            x0, x1, wxa, wxb, oh, meta = _pack_inputs(x, W, b, M=M)
            break
        except AssertionError:
            continue
    C, M, NPOOL = meta[0], meta[1], meta[2]
    nc = _get_nc(C, M, NPOOL)
    in_maps = [
        {"x0": x0[c], "x1": x1[c], "wxa": wxa[c], "wxb": wxb[c],
         "oh": np.asarray(oh[c])}
        for c in range(NCORES)
    ]
    return nc, in_maps, meta


def kernel(x, W, b):
    from concourse.bass_utils import run_bass_kernel_spmd

    nc, in_maps, meta = _prep(x, W, b)
    res = run_bass_kernel_spmd(nc, in_maps, core_ids=list(range(NCORES)))
    M = meta[1]
    pool_stats = np.stack([res.results[c]["pool_stats"] for c in range(NCORES)])
    pe_stats = np.stack([
        np.concatenate([res.results[c]["pe_y"][0:M],
                        res.results[c]["pe_y"][64:64 + M]], axis=-1)
        for c in range(NCORES)])
    return _epilogue(pool_stats.astype(np.float64),
                     pe_stats.astype(np.float64), meta)


def sim_prep(x, W, b):
    """Hook for sim_time.py: returns (nc, in_maps); caches meta for sim_check."""
    nc, in_maps, meta = _prep(x, W, b)
    _cache["meta"] = meta
    return nc, in_maps


def sim_check(sim, ins, expected):
    """Hook for sim_time.py: rel err over batches fully on core 0."""
    meta = _cache["meta"]
    C, M, NPOOL, pool_map, group_map, n = meta
    pool_stats = np.zeros((NCORES, NPOOL, 2 * D), np.float64)
    pe_stats = np.zeros((NCORES, M, 2 * D), np.float64)
    pool_stats[0] = np.asarray(sim.tensor("pool_stats")).astype(np.float64)
    _pe = np.asarray(sim.tensor("pe_y")).astype(np.float64)
    pe_stats[0] = np.concatenate([_pe[0:M], _pe[64:64 + M]], axis=-1)
    out = _epilogue(pool_stats, pe_stats, meta)
    # batches entirely inside core 0's token window
    bs = sorted(set(int(v) for v in pool_map[0] if v >= 0)
                | set(int(v) for v in group_map[0] if v >= 0))
    others = set(int(v) for v in pool_map[1:].reshape(-1) if v >= 0) \
        | set(int(v) for v in group_map[1:].reshape(-1) if v >= 0)
    bs = [b_ for b_ in bs if b_ not in others]
    return np.abs(out[bs] - expected[bs]).max() / np.abs(expected).max()
